# revision 12
# baseline (speedup 1.0000x reference)
"""Trainium2 Bass kernel for nn_DecoderMixer (L=13, B=4, T=1024, C=1024, H=16).

Sharding: data-parallel over the fused B*T axis — 8 cores x 512 rows.
Each row's 13-token attention is independent; weights replicated.

Device-side algorithm (per core, per 128-row chunk):
  - RoPE is folded into the weights HOST-side (RoPE is a linear map on the
    head dim): Wk_l = R_l @ Wk for l = 0..12 (streamed per l), and
    Wq' = R_12 @ Wq (only the last query position is ever used, since the
    module returns out[:, -1, :]).
  - K/V/Q projections run as bf16 matmuls (1 cycle/row, same PE speed as
    fp32r but half the DMA/SBUF traffic; ~2e-3 end-to-end error).
    Measured on HW: fp8 DoubleRow gives 2x MACs/cycle but the 3-term
    hi/lo compensation needed to stay accurate costs 1.5x — a net loss —
    and uncompensated fp8 is ~5e-2 error, over budget. bf16 is optimal.
  - Online attention over l (l=12 first, reusing the Q-phase x tiles):
    scores = reduce_d(q * K_l) (DVE), e = exp(s/sqrt(D)) (ACT),
    num += e * V_l (Pool/GpSimd). K/V PSUM tiles are copied to bf16 SBUF
    on the Scalar engine immediately so the PE never waits on consumers.
  - Normalization (den reduce / recip / att mul on DVE) is hoisted into
    the last-l iterations, and the att transpose + O projection for chunk
    ch-1/ch-2 are interleaved under the remaining K/V matmuls.
"""

import numpy as np
import ml_dtypes

import concourse.tile as tile
from concourse import bacc, mybir

L, B, T, C = 13, 4, 1024, 1024
H, D = 16, 64
N_CORES = 8
NPC = (B * T) // N_CORES   # 512 rows per core
CHUNK = 128
NCHUNK = NPC // CHUNK      # 4
CI = C // 128              # 8 contraction tiles
NPAIR = CI // 2            # 4 DoubleRow pairs
ROPE_BASE = 10000.0
SSCALE = 0.125             # 1/sqrt(D), folded into the Exp activation

F32 = mybir.dt.float32
BF16 = mybir.dt.bfloat16
F32R = mybir.dt.float32r
BF16NP = ml_dtypes.bfloat16

_CACHED_NC = None
_CACHED_RUNNER = None


def _proj(nc, out_ps, x, w, cs):
    """out_ps[128, C] = x[:, :, cs].T @ w over all CI contraction tiles."""
    for half in range(2):
        hs = slice(half * 512, (half + 1) * 512)
        for g in range(CI):
            nc.tensor.matmul(out_ps[:, hs], x[:, g, cs], w[:, g, hs],
                             start=(g == 0), stop=(g == CI - 1))


def _emit(tc, aps):
    nc = tc.nc
    xt, wkt, wvt, wqt = aps["xt"], aps["wkt"], aps["wvt"], aps["wqt"]
    wot, ident, xq, out = aps["wot"], aps["ident"], aps["xq"], aps["out"]

    with (
        tc.tile_pool(name="wk8", bufs=2) as wk_pool,
        tc.tile_pool(name="x8", bufs=2) as x_pool,
        tc.tile_pool(name="res", bufs=1) as res_pool,
        tc.tile_pool(name="wo", bufs=1) as wo_pool,
        tc.tile_pool(name="small", bufs=4) as small_pool,
        tc.tile_pool(name="kv", bufs=2) as kv_pool,
        tc.tile_pool(name="p", bufs=2) as p_pool,
        tc.tile_pool(name="att", bufs=4) as att_pool,
        tc.tile_pool(name="o", bufs=2) as o_pool,
        tc.tile_pool(name="ps", bufs=4, space="PSUM") as ps_pool,
    ):
        # ---- Q phase: weights + x12 first so the PE starts ASAP ----
        wq_sb = wk_pool.tile([128, CI, C], BF16, tag="w")
        x12_sb = x_pool.tile([128, CI, NPC], BF16, tag="x")
        # per-pair DMA interleave: the j-th pair of contraction tiles of
        # both Q-phase operands lands before pair j+1, so the PE starts
        # after ~0.75MB instead of 3MB.
        wqs = wqt.rearrange("(j p) c -> p j c", p=256)
        xs = xt[L - 1].rearrange("(j p) n -> p j n", p=256)
        for j in range(NPAIR):
            pj = slice(2 * j, 2 * j + 2)
            nc.sync.dma_start(
                wq_sb[:, pj, :],
                wqs[:, j, :].rearrange("(g p) c -> p g c", p=128))
            nc.sync.dma_start(
                x12_sb[:, pj, :],
                xs[:, j, :].rearrange("(g p) n -> p g n", p=128))

        # ---- resident tensors (DMAs queued behind the Q-phase ones) ----
        wv_sb = res_pool.tile([128, CI, C], BF16, tag="wv")
        nc.sync.dma_start(wv_sb[:], wvt.rearrange("(g p) c -> p g c", p=128))

        q_sb = res_pool.tile([128, NCHUNK, C], F32, tag="q")
        num_sb = res_pool.tile([128, NCHUNK, H, D], F32, tag="num")
        e_all = res_pool.tile([128, NCHUNK, L, H], F32, tag="e_all")
        nc.gpsimd.memset(num_sb[:], 0.0)

        for ch in range(NCHUNK):
            q_ps = ps_pool.tile([128, C], F32, tag="kv")
            cs = slice(ch * CHUNK, (ch + 1) * CHUNK)
            _proj(nc, q_ps, x12_sb, wq_sb, cs)
            nc.scalar.copy(q_sb[:, ch, :], q_ps[:])

        # ---- online attention over l (l=12 first: its x is resident) ----
        # AV update runs on Pool, delayed one chunk-iteration so it never
        # waits on the ACT exp round-trip.
        prev = None  # (v_sb_tile, ch, l) whose e is already requested

        att_tiles = [None] * NCHUNK
        tps_tiles = [None] * NCHUNK
        attT_tiles = [None] * NCHUNK

        def emit_T(ch):
            att2 = att_tiles[ch][:].rearrange("p h d -> p (h d)")
            t_ps = ps_pool.tile([128, C], F32, tag="kv", name=f"t_ps_{ch}")
            for g in range(CI):
                nc.tensor.transpose(
                    t_ps[:, g * 128:(g + 1) * 128],
                    att2[:, g * 128:(g + 1) * 128],
                    id_sb[:],
                )
            attT = o_pool.tile([128, CI, 128], F32R, tag="attT",
                               name=f"attT_{ch}")
            nc.scalar.copy(attT[:].rearrange("p g n -> p (g n)"), t_ps[:])
            tps_tiles[ch] = t_ps
            attT_tiles[ch] = attT

        def emit_O(ch):
            attT = attT_tiles[ch]
            o_ps = ps_pool.tile([128, C], F32, tag="kv", name=f"o_ps_{ch}")
            for half in range(2):
                hs = slice(half * 512, (half + 1) * 512)
                for g in range(CI):
                    nc.tensor.matmul(
                        o_ps[:, hs], attT[:, g, :], wo_sb[:, g, hs],
                        start=(g == 0), stop=(g == CI - 1),
                    )
            out_sb = o_pool.tile([128, C], F32, tag="out", name=f"out_{ch}")
            nc.scalar.copy(out_sb[:], o_ps[:])
            nc.sync.dma_start(out[ch * CHUNK:(ch + 1) * CHUNK, :], out_sb[:])

        def flush_prev():
            v_prev, chp, lp = prev
            m_sb = p_pool.tile([128, H, D], F32, tag="m", name=f"m_{chp}_{lp}")
            nc.gpsimd.tensor_mul(
                m_sb[:],
                v_prev[:].rearrange("p (h d) -> p h d", d=D),
                e_all[:, chp, lp, :].unsqueeze(2).broadcast_to((128, H, D)),
            )
            nc.gpsimd.tensor_add(num_sb[:, chp], num_sb[:, chp], m_sb[:])
            if lp == L - 2:
                # num/e for chunk chp are final: normalize now (DVE) so the
                # PE's epilogue transposes find att ready the moment the
                # K/V matmul stream ends.
                den = small_pool.tile([128, H], F32, tag="den")
                nc.vector.tensor_reduce(
                    den[:],
                    e_all[:, chp].transpose([0, 2, 1]),
                    axis=mybir.AxisListType.X, op=mybir.AluOpType.add,
                )
                rden = small_pool.tile([128, H], F32, tag="rd")
                nc.vector.reciprocal(rden[:], den[:])
                att_sb = att_pool.tile([128, H, D], F32, tag="att")
                nc.vector.tensor_mul(
                    att_sb[:], num_sb[:, chp],
                    rden[:].unsqueeze(2).broadcast_to((128, H, D)),
                )
                att_tiles[chp] = att_sb

        for idx, l in enumerate([L - 1] + list(range(L - 1))):
            if idx == 0:
                x_sb = x12_sb
            else:
                x_sb = x_pool.tile([128, CI, NPC], BF16, tag="x")
                nc.sync.dma_start(
                    x_sb[:], xt[l].rearrange("(g p) n -> p g n", p=128))
            wk_sb = wk_pool.tile([128, CI, C], BF16, tag="w")
            nc.sync.dma_start(
                wk_sb[:], wkt[l].rearrange("(g p) c -> p g c", p=128))
            if idx == 2:
                # epilogue-only tensors, prefetched once the first two
                # l-iterations' weights are queued.
                wo_sb = wo_pool.tile([128, CI, C], F32R, tag="w")
                nc.sync.dma_start(
                    wo_sb[:], wot.rearrange("(g p) c -> p g c", p=128))
                id_sb = res_pool.tile([128, 128], F32, tag="id")
                nc.sync.dma_start(id_sb[:], ident[:])
                # consume xq so the timing harness' rep-chaining survives
                xq_scratch = res_pool.tile([128, 16], F32, tag="xqs")
                nc.sync.dma_start(xq_scratch[:], xq[0:128, 0:16])

            for ch in range(NCHUNK):
                cs = slice(ch * CHUNK, (ch + 1) * CHUNK)
                k_ps = ps_pool.tile([128, C], F32, tag="kv")
                v_ps = ps_pool.tile([128, C], F32, tag="kv")
                _proj(nc, k_ps, x_sb, wk_sb, cs)
                k_sb = kv_pool.tile([128, C], BF16, tag="k")
                nc.scalar.copy(k_sb[:], k_ps[:])
                _proj(nc, v_ps, x_sb, wv_sb, cs)
                v_sb = kv_pool.tile([128, C], BF16, tag="v")
                nc.scalar.copy(v_sb[:], v_ps[:])

                # scores: s[n, h] = sum_d q64[n, h, d] * k64[n, h, d]
                p_sb = p_pool.tile([128, H, D], F32, tag="p")
                nc.vector.tensor_mul(
                    p_sb[:],
                    q_sb[:, ch, :].rearrange("p (h d) -> p h d", d=D),
                    k_sb[:].rearrange("p (h d) -> p h d", d=D),
                )
                s_sb = small_pool.tile([128, H], F32, tag="s")
                nc.vector.tensor_reduce(
                    s_sb[:], p_sb[:], axis=mybir.AxisListType.X,
                    op=mybir.AluOpType.add,
                )
                nc.scalar.activation(
                    e_all[:, ch, l, :], s_sb[:],
                    mybir.ActivationFunctionType.Exp, scale=SSCALE,
                )
                if prev is not None:
                    flush_prev()
                prev = (v_sb, ch, l)
                if l == L - 2:
                    # chunk ch-1's att was just computed (flush above);
                    # overlap its transpose + O projection under the
                    # remaining K/V iterations.
                    if ch >= 1:
                        emit_T(ch - 1)
                    if ch >= 2:
                        emit_O(ch - 2)
        flush_prev()
        emit_T(NCHUNK - 1)
        emit_O(NCHUNK - 2)
        emit_O(NCHUNK - 1)



def _build_bass(nrep=1):
    nc = bacc.Bacc("TRN2", target_bir_lowering=False, debug=False,
                   num_devices=N_CORES)
    aps = {
        "xt": nc.dram_tensor("xt", (L, C, NPC), BF16, kind="ExternalInput").ap(),
        "xq": nc.dram_tensor("xq", (C, NPC), F32, kind="ExternalInput").ap(),
        "wkt": nc.dram_tensor("wkt", (L, C, C), BF16, kind="ExternalInput").ap(),
        "wvt": nc.dram_tensor("wvt", (C, C), BF16, kind="ExternalInput").ap(),
        "wqt": nc.dram_tensor("wqt", (C, C), BF16, kind="ExternalInput").ap(),
        "wot": nc.dram_tensor("wot", (C, C), F32R, kind="ExternalInput").ap(),
        "ident": nc.dram_tensor("ident", (128, 128), F32, kind="ExternalInput").ap(),
    }
    if nrep == 1:
        out = nc.dram_tensor("out", (NPC, C), F32, kind="ExternalOutput").ap()
        outs = [out]
    else:
        big = nc.dram_tensor("out", (nrep, NPC, C), F32,
                             kind="ExternalOutput").ap()
        outs = [big[r] for r in range(nrep)]
    with tile.TileContext(nc) as tc:
        for r in range(nrep):
            _emit(tc, {**aps, "out": outs[r]})
    nc.compile()
    return nc


def _rope_tables():
    inv_freq = 1.0 / (ROPE_BASE ** (np.arange(0, D, 2, dtype=np.float32) / D))
    freqs = np.arange(L, dtype=np.float32)[:, None] * inv_freq[None, :]
    emb = np.concatenate([freqs, freqs], axis=-1)          # (L, D)
    return np.cos(emb).astype(np.float32), np.sin(emb).astype(np.float32)


def _rope_weight(w, cos_l, sin_l):
    """R_l @ W for a (C, C) projection weight (rows indexed by h*D+d)."""
    w3 = w.reshape(H, D, C)
    rot = np.concatenate([-w3[:, D // 2:, :], w3[:, :D // 2, :]], axis=1)
    return (cos_l[None, :, None] * w3 + sin_l[None, :, None] * rot).reshape(C, C)


def _host_prep(layer_outputs, Wq, Wk, Wv, Wo):
    cos, sin = _rope_tables()
    wkt = np.empty((L, C, C), dtype=BF16NP)
    for l in range(L):
        wkt[l] = np.ascontiguousarray(
            _rope_weight(Wk, cos[l], sin[l]).T).astype(BF16NP)
    shared = {
        "wkt": wkt,
        "wvt": np.ascontiguousarray(Wv.T).astype(BF16NP),
        "wqt": np.ascontiguousarray(
            _rope_weight(Wq, cos[L - 1], sin[L - 1]).T).astype(BF16NP),
        "wot": np.ascontiguousarray(Wo.T),
        "ident": np.eye(128, dtype=np.float32),
    }
    # x in bf16 once (2 bytes/elem), then slice/transpose per core.
    x16 = layer_outputs.astype(BF16NP)                         # (L,B,T,C)
    in_maps = []
    for c in range(N_CORES):
        n0 = c * NPC
        b = n0 // T
        t0 = n0 % T
        sl = x16[:, b, t0:t0 + NPC, :]                     # (L, NPC, C)
        in_maps.append({
            "xt": np.ascontiguousarray(sl.transpose(0, 2, 1)),
            "xq": np.zeros((C, NPC), dtype=np.float32),
            **shared,
        })
    return in_maps


def _get_nc():
    global _CACHED_NC
    if _CACHED_NC is None:
        _CACHED_NC = _build_bass()
    return _CACHED_NC


def _make_runner(nc):
    """Compile-once PJRT runner for the 8-core SPMD NEFF."""
    import jax
    from jax.experimental.shard_map import shard_map
    from jax.sharding import Mesh, NamedSharding, PartitionSpec
    from concourse.bass2jax import (
        _bass_exec_p, install_neuronx_cc_hook, partition_id_tensor,
    )

    install_neuronx_cc_hook()
    partition_name = (nc.partition_id_tensor.name
                      if nc.partition_id_tensor else None)
    in_names, out_names, out_avals, zero_outs = [], [], [], []
    for alloc in nc.m.functions[0].allocations:
        if not isinstance(alloc, mybir.MemoryLocationSet):
            continue
        name = alloc.memorylocations[0].name
        if alloc.kind == "ExternalInput":
            if name != partition_name:
                in_names.append(name)
        elif alloc.kind == "ExternalOutput":
            shape = tuple(alloc.tensor_shape)
            dtype = mybir.dt.np(alloc.dtype)
            out_names.append(name)
            out_avals.append(jax.core.ShapedArray(shape, dtype))
            zero_outs.append(np.zeros(shape, dtype))
    n_params = len(in_names)
    all_in_names = list(in_names) + list(out_names)
    if partition_name is not None:
        all_in_names.append(partition_name)

    def _body(*args):
        operands = list(args)
        if partition_name is not None:
            operands.append(partition_id_tensor())
        return tuple(_bass_exec_p.bind(
            *operands,
            out_avals=tuple(out_avals),
            in_names=tuple(all_in_names),
            out_names=tuple(out_names),
            lowering_input_output_aliases=(),
            sim_require_finite=True,
            sim_require_nnan=True,
            nc=nc,
        ))

    devices = jax.devices()[:N_CORES]
    mesh = Mesh(np.asarray(devices), ("core",))
    spec = NamedSharding(mesh, PartitionSpec("core"))
    n_outs = len(out_names)
    jitted = jax.jit(
        shard_map(_body, mesh=mesh,
                  in_specs=(PartitionSpec("core"),) * (n_params + n_outs),
                  out_specs=(PartitionSpec("core"),) * n_outs,
                  check_rep=False),
        keep_unused=True,
    )

    def run(in_maps):
        import jax as _jax
        concat_in = [
            np.concatenate([np.asarray(in_maps[c][nm])
                            for c in range(N_CORES)], axis=0)
            for nm in in_names
        ]
        dev_in = [_jax.device_put(a, spec) for a in concat_in]
        zs = [_jax.device_put(
                  np.zeros((N_CORES * z.shape[0], *z.shape[1:]), z.dtype),
                  spec)
              for z in zero_outs]
        outs = jitted(*dev_in, *zs)
        _jax.block_until_ready(outs)
        full = np.asarray(outs[out_names.index("out")])
        return full  # (N_CORES*NPC, C)

    return run


def _get_runner():
    global _CACHED_RUNNER
    if _CACHED_RUNNER is None:
        _CACHED_RUNNER = _make_runner(_get_nc())
    return _CACHED_RUNNER


def kernel(layer_outputs, Wq, Wk, Wv, Wo):
    layer_outputs = np.asarray(layer_outputs, dtype=np.float32)
    Wq = np.asarray(Wq, dtype=np.float32)
    Wk = np.asarray(Wk, dtype=np.float32)
    Wv = np.asarray(Wv, dtype=np.float32)
    Wo = np.asarray(Wo, dtype=np.float32)

    in_maps = _host_prep(layer_outputs, Wq, Wk, Wv, Wo)
    full = _get_runner()(in_maps)           # (B*T, C)
    return full.reshape(B, T, C)


if __name__ == "__main__":
    nc = _build_bass()
    print("build OK")
